# revision 1
# baseline (speedup 1.0000x reference)
"""Dice loss kernel for Trainium2 (8 NeuronCores, SPMD data-parallel).

Problem: nn_DiceLoss — logits [8,19,512,512] f32, targets [8,512,512] int64.
  probs = softmax(logits, axis=1)
  PS[c] = sum_px probs[c,px]            (probs_sum)
  I[c]  = sum_px probs[t(px),px]*[t==c] (intersection)
  CT[c] = histogram(targets)            (counts; computed on host)
  dice  = (2I+1)/(PS+CT+1); loss = mean(1-dice)

Sharding: batch b -> core b. Each core reduces its [19, 512*512] slice to
per-class partials; host combines the 8 partial vectors and finishes.

Device layout (per core): logits viewed as [19*256 rows, 1024]; a tile is 128
consecutive rows = half of one class plane -> every DMA is a contiguous
512KB 2D load (spreads across all 16 SDMA engines; strided/3D dynamic DMAs
pin to engine 0 at ~26GB/s on this runtime — measured).

Per image-half h (128 pixel-blocks, partition-aligned across all tiles):
  - ACT: E_c = exp(L_c) f32->bf16                       (19 tiles)
  - PE:  S = sum_c E_c via identity-matmul PSUM accumulation
  - DVE: r = approx-recip(S) f32, cast bf16 (r is partition-aligned with E,
         so NO broadcast is needed anywhere)
  - DVE per class: M = (T==c) [tensor_scalar 4x], W = E*r [TT 2x],
         OW = M*W [TT 2x]
  - PE:  PS[c] += colsum(W), I[c] += colsum(OW) via ones-column lhsT into
         packed [19, 1024] PSUM accumulators (one accumulation group each)
  - DVE: final [19,1024] -> [19,1] reduces; tiny DMA out.
Outputs per core: out [2,32] f32: row0 = PS[19], row1 = I[19].
"""

import functools
import sys

import numpy as np

sys.path.insert(0, "/opt/trn_rl_repo")

import ml_dtypes  # noqa: E402

B, C, H, W = 8, 19, 512, 512
HW = H * W  # 262144
F = 512  # pixels per row-block
G_MAIN = 6  # groups per main tile -> 114 partitions
G_REM = 4  # groups in remainder tile -> 76 partitions
PXT = G_MAIN * F  # 6144 pixels per main tile
N_MAIN = 42  # main tiles (42*6144 = 258048)
REM_PX0 = N_MAIN * PXT  # 258048
BATCH = 21  # tiles per reciprocal batch ([126, F] PSUM stack)
P_MAIN = C * G_MAIN  # 114
P_REM = C * G_REM  # 76
SMOOTH = 1.0
IGNORE_INDEX = 255

_CACHE = {}


ROWS = C * (HW // F)  # 9728 rows of the [row, 512] view of logits
N_H = 4  # pixel windows (128 row-blocks each)
CONST_COLS = 128 + C * C  # identity + 19 ones-column variants


def _host_consts():
    """identity [128,128] + per-class ones-column lhsT variants [128,19]."""
    bf16 = ml_dtypes.bfloat16
    cb = np.zeros((128, CONST_COLS), dtype=bf16)
    cb[:, 0:128] = np.eye(128, dtype=bf16)
    for c in range(C):
        cb[:, 128 + C * c + c] = 1  # onescol_c: column c all-ones
    return (cb,)


def _build_program():
    import concourse.bacc as bacc
    import concourse.mybir as mybir
    import concourse.tile as tile

    dt = mybir.dt
    AOP = mybir.AluOpType
    ACTF = mybir.ActivationFunctionType

    nc = bacc.Bacc("TRN2", target_bir_lowering=False, debug=False)
    logits_d = nc.declare_dram_parameter("logits", [ROWS, F], dt.bfloat16, isOutput=False)
    masks_d = nc.declare_dram_parameter("masks", [ROWS, F], dt.bfloat16, isOutput=False)
    cb_d = nc.declare_dram_parameter("consts_bf", [128, CONST_COLS], dt.bfloat16, isOutput=False)
    out_d = nc.declare_dram_parameter("out", [2, 32], dt.float32, isOutput=True)

    with tile.TileContext(nc) as tc:
        with (
            tc.tile_pool(name="singles", bufs=1) as sing,
            tc.tile_pool(name="Lp", bufs=4) as Lp,
            tc.tile_pool(name="Ep", bufs=22) as Ep,
            tc.tile_pool(name="Tp", bufs=4) as Tp,
            tc.tile_pool(name="Rp", bufs=2) as Rp,
            tc.tile_pool(name="Mp", bufs=2) as Mp,
            tc.tile_pool(name="Wp", bufs=2) as Wp,
            tc.tile_pool(name="psS", bufs=2, space="PSUM") as psS,
            tc.tile_pool(name="psAcc", bufs=1, space="PSUM") as psAcc,
        ):
            consts = sing.tile([128, CONST_COLS], dt.bfloat16)
            nc.sync.dma_start(consts[:], cb_d[:])
            ident = consts[0:128, 0:128]
            onescol = [consts[0:128, 128 + C * c : 128 + C * (c + 1)] for c in range(C)]

            psAll = psAcc.tile([C, 2 * F], dt.float32, tag="acc")  # [:, :F]=PS, [:, F:]=I

            for h in range(N_H):
                SP = psS.tile([128, F], dt.float32, tag="S")
                Es = []
                for c in range(C):
                    r0 = c * (HW // F) + 128 * h
                    L = Lp.tile([128, F], dt.bfloat16, tag="L")
                    nc.sync.dma_start(L[:], logits_d[r0 : r0 + 128, :])
                    E = Ep.tile([128, F], dt.bfloat16, tag="E")
                    nc.scalar.activation(E[:], L[:], ACTF.Exp)
                    Es.append(E)
                    nc.tensor.matmul(
                        SP[:], ident, E[:], start=(c == 0), stop=(c == C - 1)
                    )
                Rf = Rp.tile([128, F], dt.float32, tag="Rf")
                nc.vector.reciprocal_approx_fast(Rf[:], SP[:])
                Rb = Rp.tile([128, F], dt.bfloat16, tag="Rb")
                nc.vector.tensor_copy(Rb[:], Rf[:])

                for c in range(C):
                    r0 = c * (HW // F) + 128 * h
                    M = Mp.tile([128, F], dt.bfloat16, tag="M")
                    nc.sync.dma_start(M[:], masks_d[r0 : r0 + 128, :])
                    # W and OW side by side in one tile: cols [0:F]=W, [F:2F]=OW
                    WOW = Wp.tile([128, 2 * F], dt.bfloat16, tag="W")
                    nc.vector.tensor_tensor(
                        out=WOW[:, 0:F], in0=Es[c][:], in1=Rb[:], op=AOP.mult
                    )
                    nc.vector.tensor_tensor(
                        out=WOW[:, F : 2 * F], in0=M[:], in1=WOW[:, 0:F], op=AOP.mult
                    )
                    first = h == 0 and c == 0
                    last = h == N_H - 1 and c == C - 1
                    for j in range(2):
                        nc.tensor.matmul(
                            psAll[:, j * F : (j + 1) * F],
                            onescol[c],
                            WOW[:, j * F : (j + 1) * F],
                            start=first,
                            stop=last,
                        )

            psv = sing.tile([C, 1], dt.float32)
            iv = sing.tile([C, 1], dt.float32)
            nc.vector.tensor_reduce(
                psv[:], psAll[:, 0:F], axis=mybir.AxisListType.X, op=AOP.add
            )
            nc.vector.tensor_reduce(
                iv[:], psAll[:, F : 2 * F], axis=mybir.AxisListType.X, op=AOP.add
            )
            nc.sync.dma_start(out_d[0:1, 0:C], psv[:])
            nc.sync.dma_start(out_d[1:2, 0:C], iv[:])

    nc.compile()
    return nc


def _get_program():
    if "nc" not in _CACHE:
        _CACHE["nc"] = _build_program()
        _CACHE["consts"] = _host_consts()
    return _CACHE["nc"], _CACHE["consts"]


def _install_ntff_hook():
    """antenv.axon_hooks is missing in this image; synthesize it so
    run_bass_kernel_spmd(trace=True) can capture NTFF profiles via axon."""
    import types

    if "antenv.axon_hooks" in sys.modules:
        return
    mod = types.ModuleType("antenv.axon_hooks")
    _h = [None]
    mod.set_axon_ntff_profile_hook = lambda h: _h.__setitem__(0, h)
    mod.get_axon_ntff_profile_hook = lambda: _h[0]
    sys.modules["antenv.axon_hooks"] = mod
    import antenv

    antenv.axon_hooks = mod
    from trn_agent_boot.trn_boot import _ntff_profile_via_ctypes

    mod.set_axon_ntff_profile_hook(
        _ntff_profile_via_ctypes("/opt/axon/libaxon_pjrt.so")
    )


def _run_device(logits_np, targets_np, trace=False):
    """Run the SPMD kernel on 8 cores; returns (list of out arrays, results obj)."""
    from concourse.bass_utils import run_bass_kernel_spmd

    nc, (cb,) = _get_program()
    lg = (
        np.asarray(logits_np, dtype=np.float32)
        .reshape(B, ROWS, F)
        .astype(ml_dtypes.bfloat16)
    )
    tg = np.asarray(targets_np).reshape(B, 1, HW)
    masks = (tg == np.arange(C).reshape(1, C, 1)).astype(ml_dtypes.bfloat16)
    masks = masks.reshape(B, ROWS, F)
    in_maps = [
        {"logits": lg[b], "masks": masks[b], "consts_bf": cb} for b in range(B)
    ]
    kwargs = {}
    if trace:
        _install_ntff_hook()
        kwargs = {"trace": True, "trace_cores": [0]}
    res = run_bass_kernel_spmd(nc, in_maps, core_ids=list(range(B)), **kwargs)
    outs = [res.results[b]["out"] for b in range(B)]
    return outs, res


def _combine(outs, targets_np):
    PS = np.zeros(C, dtype=np.float64)
    I = np.zeros(C, dtype=np.float64)
    for o in outs:
        PS += o[0, :C].astype(np.float64)
        I += o[1, :C].astype(np.float64)
    t = np.asarray(targets_np).reshape(-1)
    valid = t != IGNORE_INDEX
    if not valid.any():
        return np.asarray(0.0, dtype=np.float32)
    CT = np.bincount(t[valid].astype(np.int64), minlength=C).astype(np.float64)
    dice = (2.0 * I + SMOOTH) / (PS + CT + SMOOTH)
    loss = (1.0 - dice).mean()
    return np.asarray(loss, dtype=np.float32)


def kernel(logits, targets):
    logits = np.asarray(logits)
    targets = np.asarray(targets)
    outs, _ = _run_device(logits, targets)
    return _combine(outs, targets)

